# revision 1
# baseline (speedup 1.0000x reference)
# Trainium2 Bass kernel for DenseBipartiteGAT (B=8, N=1024, C=256, H=4, D=64).
#
# Math: scores[t,s,h] = lrelu(a_tgt[t,h] + a_src[s,h], 0.2), masked softmax over s,
#       out[t] = sum_s attn * h_src.
# Key factorization: exp(lrelu(u+v)) = exp(u)exp(v) if u+v>=0 else exp(.2u)exp(.2v).
# With M1 = edge_mask * [u+v>=0] (a 0/1 fp16 matrix) and em = edge_mask:
#   num = E1[t]*(M1^T @ F1h) + E2[t]*((em^T @ F2h) - (M1^T @ F2h))
# and dividing num/den cancels E2, leaving r[t] = exp(0.8*u[t]) as the only
# target-side scale: out = (r*A1 + G - A2) / (r*A1d + Gd - A2d + eps).
# So the only O(N^2) elementwise work is: em = (adj != 0), P = [u+v>=0],
# M1 = P*em  -- everything else is matmuls on the PE.
#
# Sharding: data-parallel over batch B across the 8 cores (1 batch element each).

import hashlib
import os
import shutil

import numpy as np

B, N, C, H, D = 8, 1024, 256, 4, 64
NT = N // 128  # 8 tiles of 128 along s or t
EPS = 1e-12

_CACHED = {}


def _install_neff_cache():
    """Content-addressed NEFF cache: walrus compile is ~8min, cache by BIR hash."""
    import concourse.bass2jax as b2j
    import concourse.bass_utils as bu

    if getattr(b2j, "_neff_cache_installed", False):
        return
    cache_dir = os.environ.get("NEFF_CACHE_DIR", "/tmp/neff_cache")
    os.makedirs(cache_dir, exist_ok=True)
    orig = bu.compile_bir_kernel

    def cached_compile(bir_json: bytes, tmpdir: str, neff_name="file.neff") -> str:
        key = hashlib.sha256(bir_json).hexdigest()
        cpath = os.path.join(cache_dir, f"{key}.neff")
        opath = os.path.join(tmpdir, neff_name)
        if os.path.exists(cpath):
            shutil.copy(cpath, opath)
            return opath
        neff = orig(bir_json, tmpdir, neff_name)
        try:
            shutil.copy(neff, cpath)
        except OSError:
            pass
        return neff

    bu.compile_bir_kernel = cached_compile
    b2j.compile_bir_kernel = cached_compile
    b2j._neff_cache_installed = True


def build_nc():
    """Build the Bass program (one core's work; SPMD across 8 cores)."""
    import concourse.bass as bass
    import concourse.tile as tile
    import concourse.mybir as mybir
    from concourse import bacc
    from concourse.bass import ts, ds

    f32 = mybir.dt.float32
    f16 = mybir.dt.float16
    Alu = mybir.AluOpType
    Act = mybir.ActivationFunctionType

    nc = bacc.Bacc("TRN2", target_bir_lowering=False, debug=False, num_devices=B)

    xsT = nc.dram_tensor("xsT", (C, N), f32, kind="ExternalInput").ap()
    xtT = nc.dram_tensor("xtT", (C, N), f32, kind="ExternalInput").ap()
    adj = nc.dram_tensor("adj", (N, N), f32, kind="ExternalInput").ap()
    maskp = nc.dram_tensor("maskp", (128, NT), f32, kind="ExternalInput").ap()
    wes = nc.dram_tensor("wes", (C, 260), f32, kind="ExternalInput").ap()
    wbt = nc.dram_tensor("wbt", (C, 4), f32, kind="ExternalInput").ap()
    biasrow = nc.dram_tensor("biasrow", (1, 256), f32, kind="ExternalInput").ap()
    out = nc.dram_tensor("out", (N, 256), f32, kind="ExternalOutput").ap()

    with tile.TileContext(nc) as tc:
        with (
            tc.tile_pool(name="singles", bufs=1) as singles,
            tc.tile_pool(name="psum", bufs=8, space="PSUM") as psum_pool,
            tc.tile_pool(name="adjs", bufs=3) as adj_pool,
            tc.tile_pool(name="em16", bufs=3) as em16_pool,
            tc.tile_pool(name="emT", bufs=NT) as emT_pool,
            tc.tile_pool(name="rtile", bufs=NT) as r_pool,
            tc.tile_pool(name="asb", bufs=NT) as a_pool,
            tc.tile_pool(name="fex", bufs=2) as f_pool,
            tc.tile_pool(name="m1", bufs=NT) as m1_pool,
            tc.tile_pool(name="pp", bufs=4) as p_pool,
            tc.tile_pool(name="comb", bufs=4) as comb_pool,
            tc.tile_pool(name="outs", bufs=NT) as out_pool,
            tc.tile_pool(name="gsbp", bufs=NT) as gsb_pool,
            tc.tile_pool(name="dram", bufs=1, space="DRAM") as dram_pool,
        ):
            # ---- constant / weight loads ----
            xsT_sb = singles.tile([128, 2, N], f32)
            nc.sync.dma_start(xsT_sb, xsT.rearrange("(ko p) n -> p ko n", p=128))
            xtT_sb = singles.tile([128, 2, N], f32)
            nc.sync.dma_start(xtT_sb, xtT.rearrange("(ko p) n -> p ko n", p=128))
            wes_sb = singles.tile([128, 2, 260], f32)
            nc.sync.dma_start(wes_sb, wes.rearrange("(ko p) n -> p ko n", p=128))
            wbt_sb = singles.tile([128, 2, 4], f32)
            nc.sync.dma_start(wbt_sb, wbt.rearrange("(ko p) n -> p ko n", p=128))
            maskp_sb = singles.tile([128, NT], f32)
            nc.sync.dma_start(maskp_sb, maskp)
            bias_bc = singles.tile([128, 256], f32)
            nc.gpsimd.dma_start(bias_bc, biasrow.broadcast_to([128, 256]))

            em_dram = dram_pool.tile([N, N], f16)

            # ---- phase A: h_src matmul + R build (source side) ----
            r_tiles = []
            a_tiles = []
            for st in range(NT):
                ps = psum_pool.tile([128, 512], f32, tag="ps")
                for ko in range(2):
                    nc.tensor.matmul(
                        ps[:, :260],
                        lhsT=xsT_sb[:, ko, ts(st, 128)],
                        rhs=wes_sb[:, ko, :],
                        start=(ko == 0),
                        stop=(ko == 1),
                    )
                a_sb = a_pool.tile([128, 4], f32)
                nc.scalar.activation(a_sb, ps[:, 256:260], Act.Identity)
                F = f_pool.tile([128, 2, 4], f32)
                nc.scalar.activation(F[:, 0, :], ps[:, 256:260], Act.Exp)
                nc.scalar.activation(F[:, 1, :], ps[:, 256:260], Act.Exp, scale=0.2)
                # fold source-side mask into the F scales (masks both num & den)
                nc.vector.tensor_scalar(
                    F[:, :, :], F[:, :, :], maskp_sb[:, st : st + 1], None, Alu.mult
                )
                R = r_pool.tile([128, 4, 130], f16)
                for h in range(4):
                    nc.scalar.activation(
                        R[:, h, 0:64], ps[:, h * 64 : (h + 1) * 64],
                        Act.Identity, scale=F[:, 0, h : h + 1],
                    )
                    nc.scalar.activation(
                        R[:, h, 65:129], ps[:, h * 64 : (h + 1) * 64],
                        Act.Identity, scale=F[:, 1, h : h + 1],
                    )
                nc.vector.tensor_copy(out=R[:, :, 64], in_=F[:, 0, :])
                nc.vector.tensor_copy(out=R[:, :, 129], in_=F[:, 1, :])
                r_tiles.append(R)
                a_tiles.append(a_sb)

            # ---- phase A2: target side (r scales + u vectors) ----
            r_sb_tiles = []
            for tt in range(NT):
                ps = psum_pool.tile([128, 512], f32, tag="ps")
                for ko in range(2):
                    nc.tensor.matmul(
                        ps[:, 0:4],
                        lhsT=xtT_sb[:, ko, ts(tt, 128)],
                        rhs=wbt_sb[:, ko, :],
                        start=(ko == 0),
                        stop=(ko == 1),
                    )
                r_sb = a_pool.tile([128, 4], f32, tag="rsb")
                nc.scalar.activation(r_sb, ps[:, 0:4], Act.Exp, scale=0.8)
                r_sb_tiles.append(r_sb)

            u_sb = singles.tile([4, N], f16)
            for half in range(2):
                ps = psum_pool.tile([128, 512], f32, tag="ps")
                for ko in range(2):
                    nc.tensor.matmul(
                        ps[0:4, 0:512],
                        lhsT=wbt_sb[:, ko, :],
                        rhs=xtT_sb[:, ko, ds(half * 512, 512)],
                        start=(ko == 0),
                        stop=(ko == 1),
                    )
                nc.scalar.activation(
                    u_sb[:, half * 512 : (half + 1) * 512], ps[0:4, 0:512], Act.Identity
                )
            u_dram = dram_pool.tile([4, N], f16)
            nc.sync.dma_start(u_dram, u_sb)
            u_pair = []
            for pp in range(2):
                up = singles.tile([128, 2, N], f16, tag=f"upair{pp}")
                for i in range(2):
                    h = 2 * pp + i
                    nc.gpsimd.dma_start(
                        out=up[:, i, :], in_=u_dram[h : h + 1, :].broadcast_to([128, N])
                    )
                u_pair.append(up)

            # ---- phase B: edge mask em = (adj != 0), transposed via DRAM ----
            for tt in range(NT):
                adj_t = adj_pool.tile([128, N], f32)
                nc.sync.dma_start(adj_t, adj[ts(tt, 128), :])
                em16 = em16_pool.tile([128, N], f16)
                nc.vector.tensor_scalar(em16, adj_t, 0.0, None, Alu.not_equal)
                nc.sync.dma_start(em_dram[ts(tt, 128), :], em16)
            emT_tiles = []
            for st in range(NT):
                emT = emT_pool.tile([128, N], f16)
                nc.sync.dma_start_transpose(emT, em_dram[:, ts(st, 128)])
                emT_tiles.append(emT)

            # ---- phase C: two head-pass pipelines ----
            g_sb_tiles = [None] * NT
            out_tiles = [
                out_pool.tile([128, 256], f32, name=f"outt{t}", tag="outt") for t in range(NT)
            ]

            for p in range(2):
                heads = (2 * p, 2 * p + 1)
                # masks M1 for this head pair, all 8 s-tiles (resident)
                m1_tiles = []
                for st in range(NT):
                    m1 = m1_pool.tile([128, 2, N], f16)
                    for i, h in enumerate(heads):
                        pt = p_pool.tile([128, N], f16)
                        nc.vector.tensor_scalar(
                            pt, u_pair[p][:, i, :],
                            a_tiles[st][:, h : h + 1], 0.0, Alu.add, Alu.is_ge,
                        )
                        nc.vector.tensor_tensor(m1[:, i, :], pt, emT_tiles[st], Alu.mult)
                    m1_tiles.append(m1)

                tgroups = [[0, 1, 2, 3], [4, 5, 6, 7]] if p == 0 else [list(range(NT))]
                for g in tgroups:
                    psm = {}
                    psg = {}
                    for t in g:
                        psm[t] = psum_pool.tile([128, 512], f32, name=f"psm{p}_{t}", tag="ps")
                        if p == 0:
                            psg[t] = psum_pool.tile([128, 512], f32, name=f"psg{t}", tag="ps")
                    for i, h in enumerate(heads):
                        for st in range(NT):
                            for t in g:
                                nc.tensor.matmul(
                                    psm[t][:, i * 130 : (i + 1) * 130],
                                    lhsT=m1_tiles[st][:, i, ts(t, 128)],
                                    rhs=r_tiles[st][:, h, :],
                                    start=(st == 0),
                                    stop=(st == NT - 1),
                                )
                    if p == 0:
                        for st in range(NT):
                            for t in g:
                                nc.tensor.matmul(
                                    psg[t][:, 0:260],
                                    lhsT=emT_tiles[st][:, ts(t, 128)],
                                    rhs=r_tiles[st][:, :, 65:130],
                                    start=(st == 0),
                                    stop=(st == NT - 1),
                                )
                    # combine per t-tile
                    for t in g:
                        if p == 0:
                            g_sb = gsb_pool.tile([128, 4, 65], f32, tag="gsb")
                            nc.scalar.activation(
                                g_sb.rearrange("p a b -> p (a b)"),
                                psg[t][:, 0:260], Act.Identity,
                            )
                            g_sb_tiles[t] = g_sb
                        V = comb_pool.tile([128, 2, 65], f32, tag="vt")
                        for i, h in enumerate(heads):
                            nc.scalar.activation(
                                V[:, i, :], psm[t][:, i * 130 : i * 130 + 65],
                                Act.Identity, scale=r_sb_tiles[t][:, h : h + 1],
                            )
                        A2 = comb_pool.tile([128, 2, 65], f32, tag="a2")
                        psm_r = psm[t][:, 0:260].rearrange("p (i c) -> p i c", i=2)
                        nc.scalar.activation(A2, psm_r[:, :, 65:130], Act.Identity)
                        W = comb_pool.tile([128, 2, 65], f32, tag="wt")
                        nc.vector.tensor_tensor(
                            W, V, g_sb_tiles[t][:, 2 * p : 2 * p + 2, :], Alu.add
                        )
                        nc.vector.tensor_tensor(W, W, A2, Alu.subtract)
                        dent = comb_pool.tile([128, 2], f32, tag="dent")
                        nc.vector.tensor_scalar(dent, W[:, :, 64], EPS, None, Alu.add)
                        nc.vector.reciprocal(dent, dent)
                        for i, h in enumerate(heads):
                            nc.vector.tensor_scalar(
                                out_tiles[t][:, h * 64 : (h + 1) * 64],
                                W[:, i, 0:64], dent[:, i : i + 1], None, Alu.mult,
                            )
                        if p == 1:
                            nc.vector.tensor_tensor(
                                out_tiles[t], out_tiles[t], bias_bc, Alu.add
                            )
                            nc.vector.tensor_scalar(
                                out_tiles[t], out_tiles[t],
                                maskp_sb[:, t : t + 1], None, Alu.mult,
                            )
                            nc.sync.dma_start(out[ts(t, 128), :], out_tiles[t])

    nc.compile()
    return nc


def host_prep(x_source, x_target, adj, mask, W_src, W_tgt, att_src, att_tgt, bias):
    """Per-core input maps (layout prep only: transposes / weight folding)."""
    x_source = np.asarray(x_source, dtype=np.float32)
    x_target = np.asarray(x_target, dtype=np.float32)
    adj = np.ascontiguousarray(np.asarray(adj, dtype=np.float32))
    mask = np.asarray(mask)
    W_src = np.asarray(W_src, dtype=np.float32)
    W_tgt = np.asarray(W_tgt, dtype=np.float32)
    att_src = np.asarray(att_src, dtype=np.float32)
    att_tgt = np.asarray(att_tgt, dtype=np.float32)
    bias = np.asarray(bias, dtype=np.float32)

    w_a = np.einsum(
        "hdc,hd->ch", W_src.astype(np.float64).reshape(H, D, C), att_src.astype(np.float64)
    ).astype(np.float32)
    w_b = np.einsum(
        "hdc,hd->ch", W_tgt.astype(np.float64).reshape(H, D, C), att_tgt.astype(np.float64)
    ).astype(np.float32)
    wes = np.ascontiguousarray(np.concatenate([W_src.T, w_a], axis=1))  # (256, 260)
    wbt = np.ascontiguousarray(w_b)  # (256, 4)
    biasrow = np.ascontiguousarray(bias.reshape(1, 256))

    in_maps = []
    for b in range(B):
        maskp = (
            mask[b].astype(np.float32).reshape(NT, 128).T.copy()
        )  # (128, NT), p-inner
        in_maps.append(
            {
                "xsT": np.ascontiguousarray(x_source[b].T),
                "xtT": np.ascontiguousarray(x_target[b].T),
                "adj": adj[b],
                "maskp": maskp,
                "wes": wes,
                "wbt": wbt,
                "biasrow": biasrow,
            }
        )
    return in_maps


def get_nc():
    if "nc" not in _CACHED:
        _install_neff_cache()
        _CACHED["nc"] = build_nc()
    return _CACHED["nc"]


def kernel(**inputs) -> np.ndarray:
    from concourse.bass_utils import run_bass_kernel_spmd

    nc = get_nc()
    in_maps = host_prep(**inputs)
    res = run_bass_kernel_spmd(nc, in_maps, core_ids=list(range(B)))
    return np.stack([r["out"] for r in res.results]).astype(np.float32)

